# revision 86
# baseline (speedup 1.0000x reference)
"""Trainium2 Bass kernel for nn_Attn_spa (dense transformer attention with
pre-computed bias logits), SPMD over 8 NeuronCores.

Sharding: core c handles batch b = c//2 and head-half hh = c%2 (8 of 16 heads).

v2 architecture (fp8 DoubleRow + natural-orientation attention), 204.6us
baseline -> 139.5us:
  - The x-side GEMM chains (pre, Q, K, V, L) run as fp8e4 DoubleRow
    matmuls: contraction 256 per instruction (2 k-tiles), 0.5 cycles/row
    in the cost model and 2x fp8 throughput on HW. Weights are scaled x16
    on the host into e4m3's normal range; the scales unwind exactly via
    pow-of-2 ACT scale factors and a final host-side /16.
  - x-residual compensation: chains take 8 k-pairs, the last 4 multiplying
    rx8 = e4m3(x - e4m3(x)) (true scale - e4m3 subnormals carry it) by the
    same weights into the same psum group. Restores x to ~0.1% accuracy
    for free (no combine ops); without it fp8-x noise through the silu/L
    bias path pushes rel_err over the 2e-2 gate.
  - Bias-logit factorization: u = exp(score/8) * EL with EL = exp(pi/32 L),
    so the [N,N] bias is exponentiated once (8 ACT ops) and reused by all
    8 heads via one DVE multiply per (head, m-chunk). Scores/exp/attn stay
    bf16; proj is bf16 (fp8 there fails the error budget).
  - attn@V runs in natural orientation: out[n-chunk(128 part), 65] with a
    ones column in V emitting the softmax denominator as column 64 - half
    the PE cycles of the transposed form and normalization becomes a
    per-partition tensor_scalar_mul at evac. The 8 n-chunk accumulation
    groups per head pack into TWO psum banks: one zeroing start=True
    matmul per bank, start=False accumulation (skip_group_check except
    the final stop, which must clear the group flag).
  - out_nat transposes back to [d, n] via identity-moving matmuls for the
    proj chains. attnV trails the score/exp/mul stream by one m-chunk so
    the next score never queues behind a DVE-multiply wait: phase D runs
    ACT-exp-bound and gapless.
  - Heads 0-1's score+exp (ACT) interleave between the QK/L/V chains, and
    their EL-multiplies ride the front era, keeping ACT busy through the
    PE-bound front and phase D's DVE under the ACT pace.
  - One consolidated DMA per input tensor ([128, 8, F] tiles via dram-side
    rearrange; wpre split per-co so the first sigmoid isn't gated on the
    whole 1MB): HWDGE desc-gen and SP issue serialize per DMA, so count,
    not bytes, sets the lead-in.
  - Pool (GPSIMD) cannot touch PSUM on real HW (walrus rejects it; the
    sims don't check) - every psum evac lives on ACT/DVE.
Host: y[b] = (y_part(core 2b) + y_part(core 2b+1))/16 + x[b] + bproj.
Measured on the 8-core HW path: rel_err 0.0139 vs the f32 reference.
"""

import sys

sys.path.insert(0, "/opt/trn_rl_repo")

import numpy as np

B, N, C = 4, 1024, 1024
H, DH = 16, 64
NCORES = 8
CH = C // 2  # features per core in the head-sharded dim (8 heads * 64)

_cached = {}


def _build_nc():
    import concourse.bass as bass
    import concourse.mybir as mybir
    import concourse.tile as tile
    from concourse import bacc

    f32 = mybir.dt.float32
    bf16 = mybir.dt.bfloat16
    f8 = mybir.dt.float8e4
    AF = mybir.ActivationFunctionType
    ALU = mybir.AluOpType
    DR = mybir.MatmulPerfMode.DoubleRow

    nc = bacc.Bacc("TRN2", target_bir_lowering=False, debug=False)

    xt_d = nc.dram_tensor("xt8", [C, N], f8, kind="ExternalInput")
    rxt_d = nc.dram_tensor("rxt8", [C, N], f8, kind="ExternalInput")
    wpre_d = nc.dram_tensor("wpre8", [C, C], f8, kind="ExternalInput")
    wq_d = nc.dram_tensor("wq8", [C, CH], f8, kind="ExternalInput")
    wk_d = nc.dram_tensor("wk8", [C, CH], f8, kind="ExternalInput")
    wv_d = nc.dram_tensor("wv8", [C, CH], f8, kind="ExternalInput")
    wproj_d = nc.dram_tensor("wprojb", [CH, C], bf16, kind="ExternalInput")
    bpre_d = nc.dram_tensor("bpre", [C], f32, kind="ExternalInput")
    bpre16_d = nc.dram_tensor("bpre16", [C], f32, kind="ExternalInput")
    pi_d = nc.dram_tensor("pi", [1, 1], f32, kind="ExternalInput")
    id_d = nc.dram_tensor("identb", [128, 128], bf16, kind="ExternalInput")
    y_d = nc.dram_tensor("y", [N, C], bf16, kind="ExternalOutput")

    with tile.TileContext(nc) as tc:
        # ---- persistent SBUF (freed in reverse order at the end) ----
        frees = []

        def salloc(shape, dt, name):
            t, f = tc.tile(shape, dt, name=name)
            frees.append(f)
            return t

        def salloc_n(n, shape, dt, name):
            return [salloc(shape, dt, f"{name}{i}") for i in range(n)]

        ones_sb = salloc([128, 128], f32, "ones")
        pi_sb = salloc([1, 1], f32, "pisb")
        pi32_sb = salloc([128, 1], f32, "pi32")
        bpre_sb = salloc([128, 8], f32, "bpresb")
        bpre16_sb = salloc([128, 8], f32, "bpre16sb")
        ident = salloc([128, 128], bf16, "ident")
        zb = salloc([128, 260], bf16, "zb")
        xt8 = salloc([128, 8, 1024], f8, "xt8")
        rxt8 = salloc([128, 8, 1024], f8, "rxt8")
        wpre8 = salloc([128, 8, 1024], f8, "wpre8")
        wq8 = salloc([128, 8, 512], f8, "wq8")
        wk8 = salloc([128, 8, 512], f8, "wk8")
        wv8 = salloc([128, 8, 512], f8, "wv8")
        wprojb = salloc_n(4, [128, 1024], bf16, "wprojb_")
        pre8 = salloc_n(4, [128, 2, 1024], f8, "pre8_")
        qtb = salloc_n(4, [128, 1024], bf16, "qtb_")
        ktb = salloc_n(4, [128, 1024], bf16, "ktb_")
        elb = salloc_n(8, [128, 1024], bf16, "elb_")
        v_sb = salloc_n(8, [128, 520], bf16, "vsb_")
        out_natb = salloc_n(4, [128, 1024], bf16, "onat_")
        outtb = salloc_n(4, [128, 1024], bf16, "outtb_")

        nc.vector.memset(ones_sb[:], 1.0)
        nc.vector.memset(zb[:], 0.0)

        # ---- DMA. ONE consolidated DMA per input tensor ([128, 8, F] tile
        # via dram-side rearrange): HWDGE descriptor-gen (~500-625ns/DMA,
        # serialized) and SP dma_start issue (~565ns each) make DMA COUNT
        # the lead-in limiter, not bytes.
        def load8(eng, sb, d):
            eng.dma_start(sb[:, :, :], d[:, :].rearrange("(t p) n -> p t n", t=8))

        load8(nc.sync, xt8, xt_d)
        # wpre arrives per-co-column so the first pre chain (and with it the
        # first sigmoid — the ACT-era start) isn't gated on the whole 1MB
        nc.sync.dma_start(
            wpre8[:, :, 0:128],
            wpre_d[:, 0:128].rearrange("(t p) n -> p t n", t=8),
        )
        load8(nc.sync, rxt8, rxt_d)
        for co in range(1, 8):
            nc.sync.dma_start(
                wpre8[:, :, 128 * co : 128 * (co + 1)],
                wpre_d[:, 128 * co : 128 * (co + 1)].rearrange(
                    "(t p) n -> p t n", t=8
                ),
            )
        load8(nc.sync, wq8, wq_d)
        load8(nc.sync, wk8, wk_d)
        load8(nc.sync, wv8, wv_d)
        nc.gpsimd.dma_start(pi_sb[0:1, 0:1], pi_d[:, :])
        nc.gpsimd.dma_start(bpre_sb[:, :], bpre_d.rearrange("(c p) -> p c", p=128))
        nc.gpsimd.dma_start(bpre16_sb[:, :], bpre16_d.rearrange("(c p) -> p c", p=128))

        with tc.tile_pool(name="work", bufs=1) as work, \
             tc.tile_pool(name="pd", bufs=1, space="PSUM") as pd:
            pearly_cm = tc.tile_pool(name="pse", bufs=1, space="PSUM")
            pearly = pearly_cm.__enter__()

            def dr_chain(ps, stat_fn, mov_fn, nk=4):
                """Accumulate a [128,1024] psum tile (4x 256-wide DR outs, 2
                banks) over nk k-pair chunks. One start=True per BANK (the
                odd 256-slice rides start=False onto pending-zero bytes)."""
                for j in range(nk):
                    for nq in range(4):
                        nc.tensor.matmul(
                            ps[:, 256 * nq : 256 * (nq + 1)],
                            stat_fn(j), mov_fn(j, nq),
                            start=(j == 0 and nq % 2 == 0),
                            stop=(j == nk - 1 and nq % 2 == 1),
                            perf_mode=DR,
                        )

            # x-residual compensation: chains run 8 k-pairs, the last 4 being
            # W8.T @ rx8 (rx8 = e4m3(x - e4m3(x)) at true scale; e4m3
            # subnormals carry the residual). Same psum group, no combine op.
            def kp(sb, j, lo, size):
                return sb[:, 2 * j : 2 * j + 2, lo : lo + size]

            def xc_mov(j, nq):
                src = xt8 if j < 4 else rxt8
                return kp(src, j % 4, 256 * nq, 256)

            # pi/32/256 broadcast to 128 partitions via PE, first thing (the
            # ACT Copy must precede the sigmoids so it shares their table era
            # and never blocks the Exp-set load)
            pi_ps = pearly.tile([128, 1024], f32, tag="big", bufs=3, name="pips")
            nc.tensor.matmul(
                pi_ps[:, 0:1], ones_sb[0:1, 0:128], pi_sb[0:1, 0:1],
                start=True, stop=True,
            )
            nc.scalar.activation(pi32_sb[:], pi_ps[:, 0:1], AF.Copy, scale=1.0 / 8192.0)

            # ---- phase PRE: preT = silu(Wpre.T @ xT + bpre), x16 in fp8 ----
            for co in range(8):
                ps = pearly.tile([128, 1024], f32, tag="big", bufs=3)
                dr_chain(
                    ps,
                    lambda j, co=co: kp(wpre8, j % 4, 128 * co, 128),
                    xc_mov, nk=8,
                )
                # psum = 16*z ; silu(z) = z*sigmoid(z); pre8 = 16*pre
                sg = work.tile([128, 1024], f32, tag="sg", bufs=3)
                nc.scalar.activation(
                    sg[:], ps[:], AF.Sigmoid,
                    bias=bpre_sb[:, co : co + 1], scale=1.0 / 16.0,
                )
                nc.vector.scalar_tensor_tensor(
                    pre8[co // 2][:, co % 2, :], ps[:], bpre16_sb[:, co : co + 1],
                    sg[:], ALU.add, ALU.mult,
                )

            # ---- phases QK / L / V, with heads 0-1's score+exp interleaved
            # (keeps ACT fed through the PE-heavy QK/L window) ----
            def qk_unit(cq, w_sb, dst, eng):
                ps = pearly.tile([128, 1024], f32, tag="big", bufs=3)
                dr_chain(
                    ps,
                    lambda j: kp(w_sb, j % 4, 128 * cq, 128),
                    xc_mov, nk=8,
                )
                eng(dst[cq][:], ps[:])

            def l_unit(m):
                # L psum = (16 pre)@(16 pre).T = 256*L_raw; EL = exp(pi/32*L)
                ps = pearly.tile([128, 1024], f32, tag="big", bufs=3)
                dr_chain(
                    ps,
                    lambda j: pre8[j][:, :, 128 * m : 128 * (m + 1)],
                    lambda j, nq: pre8[j][:, :, 256 * nq : 256 * (nq + 1)],
                )
                nc.scalar.activation(elb[m][:], ps[:], AF.Exp, scale=pi32_sb[:, 0:1])
                els_done.add(m)

            def v_unit(nvp):
                # v natural [n, 512] x16, + ones col per head
                ps = pearly.tile([128, 1024], f32, tag="big", bufs=3)
                for half in range(2):
                    nv = 2 * nvp + half
                    for j in range(8):
                        xsrc = xt8 if j < 4 else rxt8
                        for dq in range(2):
                            nc.tensor.matmul(
                                ps[:, 512 * half + 256 * dq : 512 * half + 256 * (dq + 1)],
                                kp(xsrc, j % 4, 128 * nv, 128),
                                kp(wv8, j % 4, 256 * dq, 256),
                                start=(j == 0 and dq == 0),
                                stop=(j == 7 and dq == 1),
                                perf_mode=DR,
                            )
                # Pool cannot read PSUM on real HW: evacs go ACT/DVE
                for half in range(2):
                    nv = 2 * nvp + half
                    v3 = v_sb[nv].rearrange("p (h d) -> p h d", d=65)
                    src = ps[:, 512 * half : 512 * (half + 1)].rearrange(
                        "p (h d) -> p h d", d=64
                    )
                    if half == 0:
                        nc.scalar.copy(v3[:, :, 0:64], src)
                    else:
                        nc.vector.tensor_copy(v3[:, :, 0:64], src)
                    nc.vector.memset(v_sb[nv][:, 64::65], 1.0)

            ur_tiles = {}  # (h, m) -> exp(score) tiles awaiting the EL multiply
            ut_tiles = {}  # (h, m) -> finished attn-weight tiles (front-era muls)
            els_done = set()

            def emit_score(h, m):
                hc, hb = h // 2, (h % 2) * 64
                s = pearly.tile([128, 1024], f32, tag="searly", bufs=1)
                for half in range(2):
                    nc.tensor.matmul(
                        s[:, 512 * half : 512 * (half + 1)],
                        ktb[hc][hb : hb + 64, 128 * m : 128 * (m + 1)],
                        qtb[hc][hb : hb + 64, 512 * half : 512 * (half + 1)],
                        start=True, stop=True,
                    )
                ur = work.tile([128, 1024], bf16, tag="urp", bufs=10, name=f"ur{h}_{m}")
                nc.scalar.activation(ur[:], s[:], AF.Exp, scale=1.0 / 2048.0)
                ur_tiles[(h, m)] = ur

            qk_unit(0, wq8, qtb, nc.vector.tensor_copy)
            qk_unit(0, wk8, ktb, nc.vector.tensor_copy)
            # late inputs on HWDGE behind the 6 big loads, ahead of need
            nc.sync.dma_start(ident[:], id_d[:, :])
            for dc in range(4):
                nc.sync.dma_start(
                    wprojb[dc][:], wproj_d[128 * dc : 128 * (dc + 1), :]
                )

            units = []
            for cq in range(1, 4):
                units.append(lambda cq=cq: qk_unit(cq, wq8, qtb, nc.vector.tensor_copy))
                units.append(lambda cq=cq: qk_unit(cq, wk8, ktb, nc.vector.tensor_copy))
            # V interleaved among early L units so v_sb is ready (and its
            # evacs drained) before phase D's first attn matmuls
            for m in range(8):
                units.append(lambda m=m: l_unit(m))
                if m < 2:
                    units.append(lambda m=m: v_unit(2 * m))
                    units.append(lambda m=m: v_unit(2 * m + 1))
            pre_scores = [(h, m) for h in (0, 1) for m in range(8)]
            for i, u in enumerate(units):
                if i < len(pre_scores):
                    emit_score(*pre_scores[i])
                u()
                # heads 0-1's EL multiplies ride the front era (DVE is light
                # here; keeps phase D's DVE under the ACT exp pace)
                for key in [k for k in ur_tiles if k[0] < 2 and k[1] in els_done]:
                    ut = work.tile([128, 1024], bf16, tag="utp", bufs=16,
                                   name=f"utp{key[0]}_{key[1]}")
                    nc.vector.tensor_mul(ut[:], ur_tiles.pop(key)[:], elb[key[1]][:])
                    ut_tiles[key] = ut

            pearly_cm.__exit__(None, None, None)

            # ---- phase D: per-head attention, natural orientation ----
            with tc.tile_pool(name="pdD", bufs=1, space="PSUM") as pD:
                rec_pool = work  # [128, 8] recip tiles

                for h in range(8):
                    hc, hb = h // 2, (h % 2) * 64
                    att = [
                        pD.tile([128, 512], f32, tag="att", bufs=3, name=f"att{h}_{bk}")
                        for bk in range(2)
                    ]
                    # zero both packed banks with one start=True matmul each
                    # (stop=False: the group-started flag must stay set until
                    # the last accumulating matmul clears it)
                    for bk in range(2):
                        nc.tensor.matmul(
                            att[bk][:, 0:260], zb[:, 0:128], zb[:, 0:260],
                            start=True, stop=False,
                        )
                    def attn_mms(m, ut):
                        for n2 in range(8):
                            bk, sl = n2 // 4, n2 % 4
                            nc.tensor.matmul(
                                att[bk][:, 65 * sl : 65 * sl + 65],
                                ut[:, 128 * n2 : 128 * (n2 + 1)],
                                v_sb[m][:, 65 * h : 65 * h + 65],
                                start=False, stop=(m == 7),
                                skip_group_check=not (m == 7 and sl == 3),
                            )

                    # attnV runs one m behind the score/exp/mul stream so the
                    # next score never queues behind a matmul that waits on
                    # the DVE multiply — keeps ACT's exp stream gapless
                    pending_ut = None
                    for m in range(8):
                        if (h, m) in ut_tiles:
                            ut = ut_tiles.pop((h, m))
                        elif (h, m) in ur_tiles:
                            ut = work.tile([128, 1024], bf16, tag="ut", bufs=4)
                            nc.vector.tensor_mul(
                                ut[:], ur_tiles.pop((h, m))[:], elb[m][:]
                            )
                        else:
                            s = pD.tile([128, 1024], f32, tag="s", bufs=2)
                            for half in range(2):
                                nc.tensor.matmul(
                                    s[:, 512 * half : 512 * (half + 1)],
                                    ktb[hc][hb : hb + 64, 128 * m : 128 * (m + 1)],
                                    qtb[hc][hb : hb + 64, 512 * half : 512 * (half + 1)],
                                    start=True, stop=True,
                                )
                            ur = work.tile([128, 1024], bf16, tag="ur", bufs=4)
                            nc.scalar.activation(ur[:], s[:], AF.Exp, scale=1.0 / 2048.0)
                            ut = work.tile([128, 1024], bf16, tag="ut", bufs=4)
                            nc.vector.tensor_mul(ut[:], ur[:], elb[m][:])
                        if pending_ut is not None:
                            attn_mms(m - 1, pending_ut)
                        pending_ut = ut
                    attn_mms(7, pending_ut)
                    # normalize + evac to natural layout (pair tile)
                    rec = rec_pool.tile([128, 8], f32, tag="rec", bufs=2)
                    for bk in range(2):
                        nc.vector.reciprocal(
                            rec[:, 4 * bk : 4 * bk + 4], att[bk][:, 64:260:65]
                        )
                    for n2 in range(8):
                        bk, sl = n2 // 4, n2 % 4
                        nc.vector.tensor_scalar_mul(
                            out_natb[hc][:, 128 * n2 + 64 * (h % 2) : 128 * n2 + 64 * (h % 2) + 64],
                            att[bk][:, 65 * sl : 65 * sl + 64],
                            rec[:, n2 : n2 + 1],
                        )
                    if h % 2 == 1:
                        # both heads of pair hc done: transpose to [d, n] via
                        # identity-moving matmuls, evac to fp8 for proj
                        for q2 in range(2):
                            tr = pD.tile([128, 512], f32, tag="tr", bufs=1)
                            for k in range(4):
                                n2 = 4 * q2 + k
                                nc.tensor.matmul(
                                    tr[:, 128 * k : 128 * (k + 1)],
                                    out_natb[hc][:, 128 * n2 : 128 * (n2 + 1)],
                                    ident[:, :],
                                    start=True, stop=True,
                                )
                            nc.vector.tensor_copy(
                                outtb[hc][:, 512 * q2 : 512 * (q2 + 1)], tr[:, :]
                            )

                # ---- phase proj: y = outT.T @ Wproj (x16 total scale) ----
                for mt in range(8):
                    ps = pD.tile([128, 1024], f32, tag="s", bufs=2)
                    for dc in range(4):
                        for half in range(2):
                            nc.tensor.matmul(
                                ps[:, 512 * half : 512 * (half + 1)],
                                outtb[dc][:, 128 * mt : 128 * (mt + 1)],
                                wprojb[dc][:, 512 * half : 512 * (half + 1)],
                                start=(dc == 0), stop=(dc == 3),
                            )
                    y_sb = work.tile([128, 1024], bf16, tag="y", bufs=2)
                    eng = (nc.scalar.copy, nc.vector.tensor_copy)[mt % 2]
                    eng(y_sb[:], ps[:])
                    nc.sync.dma_start(y_d[128 * mt : 128 * (mt + 1), :], y_sb[:])

        for f in reversed(frees):
            f()

    nc.finalize()
    return nc


def get_nc():
    if "nc" not in _cached:
        _cached["nc"] = _build_nc()
    return _cached["nc"]


def core_inputs(x, Wq, Wk, Wv, Wproj, Wpre, bpre, pi, c):
    """Host-side shard + fp8 prep for core c (batch c//2, head-half c%2)."""
    import ml_dtypes

    E4 = ml_dtypes.float8_e4m3
    BF = ml_dtypes.bfloat16
    b, hh = c // 2, c % 2
    sl = slice(CH * hh, CH * (hh + 1))
    xt = np.ascontiguousarray(np.asarray(x, np.float32)[b].T)
    xt8 = xt.astype(E4)
    return {
        "xt8": xt8,
        "rxt8": (xt - xt8.astype(np.float32)).astype(E4),
        "wpre8": (np.asarray(Wpre, np.float32) * 16.0).astype(E4),
        "wq8": np.ascontiguousarray(np.asarray(Wq, np.float32)[:, sl] * 16.0).astype(E4),
        "wk8": np.ascontiguousarray(np.asarray(Wk, np.float32)[:, sl] * 16.0).astype(E4),
        "wv8": np.ascontiguousarray(np.asarray(Wv, np.float32)[:, sl] * 16.0).astype(E4),
        "wprojb": np.ascontiguousarray(np.asarray(Wproj, np.float32)[sl, :]).astype(BF),
        "bpre": np.asarray(bpre, np.float32),
        "bpre16": np.asarray(bpre, np.float32) * 16.0,
        "pi": np.asarray(pi, np.float32).reshape(1, 1),
        "identb": np.eye(128, dtype=BF),
    }


def assemble(y0, y1, xb, bproj):
    return (
        (np.asarray(y0, np.float32) + np.asarray(y1, np.float32)) * (1.0 / 16.0)
        + xb
        + np.asarray(bproj, np.float32)[None, :]
    )


def kernel(x, Wq, Wk, Wv, Wproj, bproj, Wpre, bpre, pi):
    x = np.asarray(x, np.float32)
    nc = get_nc()
    in_maps = [
        core_inputs(x, Wq, Wk, Wv, Wproj, Wpre, bpre, pi, c) for c in range(NCORES)
    ]
    from concourse.bass_utils import run_bass_kernel_spmd

    res = run_bass_kernel_spmd(nc, in_maps, list(range(NCORES)))
    y = np.empty((B, N, C), np.float32)
    for b in range(B):
        y[b] = assemble(
            res.results[2 * b]["y"], res.results[2 * b + 1]["y"], x[b],
            np.asarray(bproj, np.float32),
        )
    return y
